# revision 15
# baseline (speedup 1.0000x reference)
"""Trainium2 Bass kernel for EnhancedStrategySuperposition (MoE soft routing).

Math (per token b):
    logits = x @ W_att.T + b_att + adaptive_bias          [B, E]
    w      = softmax(logits + gumbel(u))                  [B, E]
    y[e]   = x @ W_strat[e].T + b_strat[e]                [B, E, A]
    out    = sum_e w[:, e] * y[e]                         [B, A]

Strategy:
  - Data-parallel: batch B=8192 sharded across 8 cores (1024 tokens each);
    gating + strategy weights replicated.
  - Host prep: all inputs are laid out partition-major ([128, ...] with long
    contiguous per-partition runs) so each tensor is one or a few large
    descriptor-efficient DMAs.  x and W_strat are transposed into [D, *]
    (the PE contracts along the partition dim) and cast to fp16 (full-rate
    on the PE, rel-err ~3e-4).  W_strat is grouped by expert-group so the
    first group's megabyte lands early and phase B can start while the rest
    streams.  b_att + adaptive_bias are folded into the host-computed gumbel
    noise g.
  - Phase A (gating): logits computed transposed (lhsT = W_att.T chunk,
    M=32, N=512 tokens: 16 big matmuls), PE-transposed back to token-major;
    softmax = exp (ACT, accum_out row-sum) + reciprocal + scale on DVE (no
    max-subtraction needed: logits+gumbel <= ~22).  The exact b_strat term
    sum_e w[:,e] b_strat[e,:] seeds the accumulator via a PE transpose of w
    and a K=32 matmul.
  - Phase B (strategy): per 128-token tile, expert-group-outer / d-chunk-
    inner matmuls (N=512, accumulate K=1024 in PSUM).  Group-outer staggers
    PSUM bank completion so each bank's drain (ScalarE PSUM->SBUF copy,
    then DVE scalar_tensor_tensor FMAs with per-partition scalar w[:, e])
    overlaps the next group's matmuls.  Two alternating accumulators break
    the DVE read-after-write chain; they are summed once per tile.
"""

import numpy as np

_B, _D, _E, _A = 8192, 1024, 32, 128
_NCORES = 8
_BL = _B // _NCORES  # tokens per core
_EPS = 1e-10

_KC = _D // 128  # contraction chunks
_JT = _BL // 128  # token tiles per core
_GG = _E // 4  # expert groups (4 experts x 128 cols = 512)

_cache = {}


def _build():
    """Build + compile the per-core Bass program (cached)."""
    if "nc" in _cache:
        return _cache["nc"]

    from contextlib import ExitStack

    from concourse import bacc, mybir, tile
    from concourse.bass import ts
    from concourse.masks import make_identity

    f16 = mybir.dt.float16
    f32 = mybir.dt.float32

    nc = bacc.Bacc("TRN2", debug=False, num_devices=_NCORES)

    KC, JT, GG = _KC, _JT, _GG

    xt_d = nc.dram_tensor("xt16", [128, KC * _BL], f16, kind="ExternalInput").ap()
    wt_d = nc.dram_tensor(
        "wt16", [128, GG * KC * 512], f16, kind="ExternalInput"
    ).ap()
    wa_d = nc.dram_tensor("wa16", [128, KC * _E], f16, kind="ExternalInput").ap()
    g_d = nc.dram_tensor("g32", [128, JT * _E], f32, kind="ExternalInput").ap()
    bs_d = nc.dram_tensor("bs32", [_E, _A], f32, kind="ExternalInput").ap()
    out_d = nc.dram_tensor("out", [_BL, _A], f32, kind="ExternalOutput").ap()

    with tile.TileContext(nc) as tc, ExitStack() as ctx:
        singles = ctx.enter_context(tc.tile_pool(name="singles", bufs=1))
        sb_small = ctx.enter_context(tc.tile_pool(name="small", bufs=3))

        # --- resident inputs: gating weights + x first, then W by group ---
        wabig = singles.tile([128, KC * _E], f16, tag="wabig")
        nc.sync.dma_start(out=wabig, in_=wa_d[:, :])
        xbig = singles.tile([128, KC * _BL], f16, tag="xbig")
        nc.sync.dma_start(out=xbig, in_=xt_d[:, :])
        ident = singles.tile([128, 128], f32, tag="ident")
        make_identity(nc, ident)

        wbig = singles.tile([128, GG * KC * 512], f16, tag="wbig")
        nc.sync.dma_start(out=wbig[:, ts(0, KC * 512)], in_=wt_d[:, ts(0, KC * 512)])
        g_all = singles.tile([128, JT * _E], f32, tag="g")
        nc.sync.dma_start(out=g_all, in_=g_d[:, :])
        bs_sb = singles.tile([_E, _A], f32, tag="bs")
        nc.sync.dma_start(out=bs_sb, in_=bs_d[:, :])
        for gi in range(1, GG):
            nc.sync.dma_start(
                out=wbig[:, ts(gi, KC * 512)], in_=wt_d[:, ts(gi, KC * 512)]
            )

        # --- PE warm-up: keep the HAM activity monitor busy from ~5us (as
        # soon as the GpSimd-built identity exists) until the x DMA lands,
        # so the gating + strategy matmul streams run at 2.4 GHz throughout.
        warm_sink = singles.tile([1, 1], f32, tag="warmsink")
        warm_in = singles.tile([128, 128], f32, tag="warmin")
        nc.vector.memset(warm_in, 0.25)
        with tc.tile_pool(name="pswarm", bufs=1, space="PSUM") as ps_warm:
            pw = ps_warm.tile([128, 128], f32, tag="warm")
            for _ in range(18):
                nc.tensor.matmul(pw, warm_in, warm_in, start=True, stop=True)
            nc.vector.tensor_copy(warm_sink, pw[0:1, 0:1])

        def x_lhsT(k, j):  # [128, 128] fp16, d-chunk k, token tile j
            return xbig[:, k * _BL + j * 128 : k * _BL + (j + 1) * 128]

        wsb = [
            singles.tile([128, _E], f32, tag=f"wj{j}", name=f"wj{j}")
            for j in range(JT)
        ]
        acca = [
            singles.tile([128, _A], f32, tag=f"acca{j}", name=f"acca{j}")
            for j in range(JT)
        ]
        accb = [
            singles.tile([128, _A], f32, tag=f"accb{j}", name=f"accb{j}")
            for j in range(JT)
        ]

        # ---------------- Phase A: gating for all token tiles ----------------
        # PE does only the dense lgT matmuls + logit transposes here; the
        # softmax chains run on DVE/ACT and overlap phase B's matmul stream.
        with (
            tc.tile_pool(name="plgT", bufs=1, space="PSUM") as ps_lgT,
            tc.tile_pool(name="plg", bufs=2, space="PSUM") as ps_lg,
        ):
            for jj in range(JT // 4):  # halves of 512 tokens
                plgT = ps_lgT.tile([_E, 512], f32, tag="lgT")
                for k in range(KC):
                    nc.tensor.matmul(
                        plgT,
                        wabig[:, ts(k, _E)],
                        xbig[:, k * _BL + jj * 512 : k * _BL + (jj + 1) * 512],
                        start=(k == 0),
                        stop=(k == KC - 1),
                    )
                lgT_sb = sb_small.tile([_E, 512], f32, tag="lgT_sb")
                nc.vector.tensor_copy(lgT_sb, plgT)
                for t in range(4):
                    j = jj * 4 + t
                    # logits back to token-major [128 tokens, E]
                    plg = ps_lg.tile([128, _E], f32, tag="lg")
                    nc.tensor.transpose(
                        plg, lgT_sb[:, ts(t, 128)], ident[:_E, :_E]
                    )
                    lg = sb_small.tile([128, _E], f32, tag="lgadd")
                    nc.vector.tensor_add(lg, g_all[:, ts(j, _E)], plg)
                    # softmax over E (no max-subtract needed: lg <= ~22)
                    s = sb_small.tile([128, 1], f32, tag="s")
                    nc.scalar.activation(
                        wsb[j],
                        lg,
                        mybir.ActivationFunctionType.Exp,
                        bias=0.0,
                        scale=1.0,
                        accum_out=s,
                    )
                    rinv = sb_small.tile([128, 1], f32, tag="rinv")
                    nc.vector.reciprocal(rinv, s)
                    nc.vector.tensor_scalar_mul(wsb[j], wsb[j], rinv)

        # ------------- Phase B: strategy matmuls + weighted combine -------------
        # Group-major order: each 1MB W group is consumed by all 8 token
        # tiles (13.8us of matmuls) before the next group is needed, so the
        # streaming W DMA never stalls the PE after group 0 arrives.  The
        # per-tile b_strat seed (w transpose + K=32 matmul) is emitted just
        # before the tile's first matmul block of group 0; softmax(j) is
        # long finished by then so these PE ops never stall the stream.
        with (
            tc.tile_pool(name="pswt", bufs=1, space="PSUM") as ps_wt,
            tc.tile_pool(name="psb0", bufs=1, space="PSUM") as ps_b,
            tc.tile_pool(name="psbig", bufs=6, space="PSUM") as ps_big,
            tc.tile_pool(name="ybuf", bufs=4) as ybuf,
        ):
            def emit_drain(gi, j, ps):
                if gi == 0:
                    # b_strat term: acca[j] = (w^T).T @ b_strat.  Emitted in
                    # the trailing slot so softmax(j) is guaranteed done.
                    pwt = ps_wt.tile([_E, 128], f32, tag="pwt", name="pwt")
                    nc.tensor.transpose(pwt, wsb[j], ident)
                    wt_sb = sb_small.tile([_E, 128], f32, tag="wt_sb", name="wt_sb")
                    nc.vector.tensor_copy(wt_sb, pwt)
                    pa0 = ps_b.tile([128, _A], f32, tag="pa0", name="pa0")
                    nc.tensor.matmul(pa0, wt_sb, bs_sb, start=True, stop=True)
                    nc.vector.tensor_copy(acca[j], pa0)
                ysb = ybuf.tile([128, 512], f32, tag="y", name="y")
                nc.scalar.copy(ysb, ps)
                for i in range(4):
                    e = gi * 4 + i
                    wcol = wsb[j][:, e : e + 1]
                    if e == 1:
                        # first write of the odd-chain accumulator
                        nc.vector.tensor_scalar_mul(
                            accb[j], ysb[:, ts(i, 128)], wcol
                        )
                    else:
                        dst = acca[j] if e % 2 == 0 else accb[j]
                        nc.vector.scalar_tensor_tensor(
                            out=dst,
                            in0=ysb[:, ts(i, 128)],
                            scalar=wcol,
                            in1=dst,
                            op0=mybir.AluOpType.mult,
                            op1=mybir.AluOpType.add,
                        )
                if gi == GG - 1:
                    nc.vector.tensor_add(acca[j], acca[j], accb[j])
                    nc.sync.dma_start(out=out_d[ts(j, 128), :], in_=acca[j])

            pending = None  # (gi, j, psum tile) whose drain trails one block
            for gi in range(GG):
                for j in range(JT):
                    ps = ps_big.tile([128, 512], f32, tag="bank", name="bank")
                    for k in range(KC):
                        nc.tensor.matmul(
                            ps,
                            x_lhsT(k, j),
                            wbig[:, gi * KC * 512 + k * 512 : gi * KC * 512 + (k + 1) * 512],
                            start=(k == 0),
                            stop=(k == KC - 1),
                        )
                    if pending is not None:
                        emit_drain(*pending)
                    pending = (gi, j, ps)
            emit_drain(*pending)

    nc.compile()
    _cache["nc"] = nc
    return nc


def _prep_in_maps(x, W_att, b_att, adaptive_bias, W_strat, b_strat, gumbel_u):
    x = np.asarray(x, dtype=np.float32)
    W_att = np.asarray(W_att, dtype=np.float32)
    b_att = np.asarray(b_att, dtype=np.float32)
    adaptive_bias = np.asarray(adaptive_bias, dtype=np.float32)
    W_strat = np.asarray(W_strat, dtype=np.float32)
    b_strat = np.asarray(b_strat, dtype=np.float32)
    gumbel_u = np.asarray(gumbel_u, dtype=np.float32)

    KC, JT, GG = _KC, _JT, _GG

    # x partition-major: X16[c][p, k*BL + b] = x[c*BL + b, k*128 + p]
    x16 = x.astype(np.float16)
    Xpm = x16.reshape(_B, KC, 128).transpose(2, 1, 0)  # [p, k, b_global]

    # W_strat: WT[d, e*A+a]; grouped [p, gi, k, c] with c in [0,512)
    WT = W_strat.transpose(2, 0, 1).reshape(_D, _E * _A).astype(np.float16)
    Wb = (
        WT.reshape(KC, 128, GG, 512)
        .transpose(1, 2, 0, 3)
        .reshape(128, GG * KC * 512)
    )
    Wb = np.ascontiguousarray(Wb)

    # W_att: Wa[p, k*E + e] = W_att[e, k*128+p]
    Wa = np.ascontiguousarray(
        W_att.T.astype(np.float16).reshape(KC, 128, _E).transpose(1, 0, 2)
    ).reshape(128, KC * _E)

    bias_row = (b_att + adaptive_bias).astype(np.float32)
    g = -np.log(-np.log(gumbel_u + np.float32(_EPS)) + np.float32(_EPS))
    g = (g + bias_row[None, :]).astype(np.float32)

    bs32 = np.ascontiguousarray(b_strat, dtype=np.float32)

    in_maps = []
    for c in range(_NCORES):
        sl = slice(c * _BL, (c + 1) * _BL)
        xc = np.ascontiguousarray(Xpm[:, :, sl]).reshape(128, KC * _BL)
        gc = np.ascontiguousarray(
            g[sl].reshape(JT, 128, _E).transpose(1, 0, 2)
        ).reshape(128, JT * _E)
        in_maps.append(
            {
                "xt16": xc,
                "wt16": Wb,
                "wa16": Wa,
                "g32": gc,
                "bs32": bs32,
            }
        )
    return in_maps


def kernel(x, W_att, b_att, adaptive_bias, W_strat, b_strat, gumbel_u):
    assert x.shape == (_B, _D) and W_strat.shape == (_E, _A, _D)
    nc = _build()
    in_maps = _prep_in_maps(
        x, W_att, b_att, adaptive_bias, W_strat, b_strat, gumbel_u
    )
    from concourse.bass_utils import run_bass_kernel_spmd

    try:
        res = run_bass_kernel_spmd(nc, in_maps, list(range(_NCORES))).results
    except Exception:
        # transient device errors (e.g. a wedged core from a prior run)
        # usually clear on a retry
        import time

        time.sleep(2.0)
        res = run_bass_kernel_spmd(nc, in_maps, list(range(_NCORES))).results
    out = np.concatenate([res[c]["out"] for c in range(_NCORES)], axis=0)
    return np.ascontiguousarray(out.astype(np.float32))
